# revision 40
# baseline (speedup 1.0000x reference)
"""DDGCRN cell on 8 TRN2 NeuronCores — data-parallel over batch.

Per core: 8 batches = 16 branch-instances (gate O=128 / update O=64), emitted
as a software pipeline so every engine's static instruction stream stays
dense (engines execute their streams in order; serial per-instance chains
would otherwise stall the TensorEngine and re-throttle its HAM clock gate).

Pipeline: step s emits  P6(s-2) op-matmuls+activation | P4(s-1) d-bcast+yT
| P5(s-1) Lx | P2(s) A-matmuls+relu+rowsum | P3(s) rsqrt+x' | P1(s+1)
hypernet+V.  Update(b) is sequenced >=3 slots after gate(b) (needs z).

Math per instance:
  filt = hypernet MLP (transposed-feature layout, bf16)
  V = tanh(emb*time*day*speed*occupy*filt)      (10, 883)
  A = relu(V V^T) (883,883 symmetric) + fused row-sums (ACT accum_out)
  d = rsqrt(rowsum) via fast-inverse-sqrt + 1 Newton step (DVE only; keeps
      ScalarE pinned to the sigmoid/tanh/relu table set — no table reloads)
  Lx^T = xs^T - dB2 * ((d*xs)^T A)  using A's symmetry; dB2 = GPSIMD
      partition-broadcast of the d row (128 rows: state part doubled)
  out^T = bpool.T @ embT + sum_{k,ep} wz64.T @ (embP_ep * sg2_k)
        + wz40.T @ (embX * xrep)
      (state rows packed in 64-row pairs -> 10 K=128 matmuls; the 4 x-rows
       replicated per-e via GPSIMD broadcast -> 1 K=40 matmul)

All matmuls bf16 (PSUM f32); inputs pre-cast/pre-transposed on host (pure
layout/dtype prep). Output written transposed, un-transposed on host.
"""

import sys, os

sys.path.insert(0, "/opt/trn_rl_repo")

import numpy as np
import ml_dtypes
from contextlib import ExitStack

import concourse.bass as bass
import concourse.bacc as bacc
import concourse.mybir as mybir
from concourse import tile
from concourse.alu_op_type import AluOpType
from concourse.bass_utils import run_bass_kernel_spmd

AF = mybir.ActivationFunctionType
F32 = mybir.dt.float32
BF16 = mybir.dt.bfloat16
I32 = mybir.dt.int32
BF16_NP = ml_dtypes.bfloat16

B, N, DIN, DOUT, E, CHEB = 64, 883, 2, 64, 10, 2
C = DIN + DOUT  # 66
NCORES = 8
BL = B // NCORES  # 8 batches per core
NT = (N + 127) // 128  # 7 row tiles
EKC = E * CHEB * C  # 1320
OG, OU = 2 * DOUT, DOUT  # 128, 64
SPLITS = [(0, 512), (512, N - 512)]
RSQRT_MAGIC = 0x5F3759DF

# instance schedule: update(b) >= 3 slots after gate(b)
SEQ = [("g", 0), ("g", 1), ("g", 2), ("u", 0), ("g", 3), ("u", 1), ("g", 4),
       ("u", 2), ("g", 5), ("u", 3), ("g", 6), ("u", 4), ("g", 7), ("u", 5),
       ("u", 6), ("u", 7)]


def _pt(nt):
    return min(128, N - nt * 128)


def _noload(mm):
    """Mark a matmul as reusing the PE-resident weights (skips LDWEIGHTS).

    Only valid when the immediately preceding PE-queue instruction loaded
    identical weights (the first matmul of a PSUM-split pair)."""
    mm.ins.ldweights = False
    return mm


def _build_body(tc, ctx, nc, P):
    def pool(name, bufs, space="SBUF"):
        return ctx.enter_context(tc.tile_pool(name=name, bufs=bufs, space=space))

    wp = pool("wp", 1)        # static weights
    dat = pool("dat", 2)      # per-batch DMA loads
    act = pool("act", 2)      # per-instance intermediates
    arp = pool("arp", 14)     # relu(A) tiles: 2 instances x 7 in flight
    xnp = pool("xnp", 29)     # natural xs/cand tiles
    xpp = pool("xpp", 15)     # d*xs tiles
    ztp = pool("ztp", 2)      # packed zb tiles (double buffer)
    dnp = pool("dnp", 4)      # rowsum/d helpers
    psp = pool("psp", 4, space="PSUM")  # op + yT accumulators (tag psA)
    psa = pool("psa", 4, space="PSUM")  # A halves / hypernet / transposes

    def ps_pair(p, name, parts, tag):
        return [p.tile([parts, sl], F32, tag=tag, name=f"{name}_{i}")
                for i, (s0, sl) in enumerate(SPLITS)]

    # ---------------- static setup (all bf16 direct from host) ----------
    ident_f = wp.tile([128, 128], F32, tag="identf", name="ident_f")
    nc.sync.dma_start(ident_f[:, :], P["ident"][:, :])
    ident_b = wp.tile([128, 128], BF16, tag="identb", name="ident_b")
    nc.vector.tensor_copy(ident_b[:, :], ident_f[:, :])

    def load_bf(pname, shape, tag):
        t = wp.tile(list(shape), BF16, tag=tag, name=pname + "_t")
        nc.sync.dma_start(t[:, :], P[pname][:, :])
        return t

    embT = load_bf("embT", (E, N), "embT")
    # emb broadcast tiles for the packed P6 contraction: embP[ep] rows =
    # [emb[:,2ep] x64 ; emb[:,2ep+1] x64]; embX rows (2k+c)*10+e = emb[:,e]
    embP = []
    for ep in range(5):
        t = wp.tile([128, N], BF16, tag=f"embP{ep}", name=f"embP{ep}")
        nc.sync.dma_start(t[:, :], P["embP"][ep * 128 : (ep + 1) * 128, :])
        embP.append(t)
    embX = load_bf("embX", (2 * CHEB * E, N), "embX")
    sel4 = load_bf("sel4", (2 * DIN, 2 * CHEB * E), "sel4")
    wz64 = {}
    wz40 = {}
    for br, On in (("g", OG), ("u", OU)):
        tiles = []
        for t_i in range(2 * 5):
            t = wp.tile([128, On], BF16, tag=f"wz64{br}{t_i}", name=f"wz64{br}{t_i}")
            nc.sync.dma_start(t[:, :], P[f"wz64_{br}"][t_i * 128 : (t_i + 1) * 128, :])
            tiles.append(t)
        wz64[br] = tiles
        wz40[br] = load_bf(f"wz40_{br}", (2 * CHEB * E, On), f"wz40{br}")
    fc = {}
    for br in ("g", "u"):
        fc[("w1", br)] = load_bf(f"fc1w_{br}", (C, 16), f"fc1w{br}")
        fc[("w2", br)] = load_bf(f"fc2w_{br}", (16, 2), f"fc2w{br}")
        fc[("w3", br)] = load_bf(f"fc3w_{br}", (2, E), f"fc3w{br}")
        for nm, shape in (("b1", (16, 1)), ("b2", (2, 1)), ("b3", (E, 1))):
            t = wp.tile(list(shape), F32, tag=f"fc{nm}{br}", name=f"fc{nm}{br}")
            nc.sync.dma_start(t[:, :], P[f"fc{nm}_{br}"][:, :])
            fc[(nm, br)] = t
    bp = {"g": load_bf("bpool_g", (E, OG), "bpg"),
          "u": load_bf("bpool_u", (E, OU), "bpu")}

    # ---------------- per-instance state ----------------
    ST = {}   # (br,b) -> dict of tiles
    BAT = {}  # b -> dict of per-batch tiles

    def batch_load(b):
        """DMA this batch's inputs; build Mb and gate xg2."""
        d = {}
        xs_nat = []
        for nt in range(NT):
            p = _pt(nt)
            t = xnp.tile([128, C], BF16, tag="xsn", name=f"xsn{b}{nt}")
            nc.sync.dma_start(t[:p, 0:DIN], P["x"][b, nt * 128 : nt * 128 + p, :])
            nc.sync.dma_start(t[:p, DIN:C], P["state"][b, nt * 128 : nt * 128 + p, :])
            xs_nat.append(t)
        d["xs_nat"] = xs_nat
        xsTt = act.tile([C, N], BF16, tag="xsT", name=f"xsT{b}", bufs=4)
        nc.sync.dma_start(xsTt[:, :], P["xsT"][b, :, :])
        d["xsT"] = xsTt
        xsT2t = act.tile([128, N], BF16, tag="xsT2", name=f"xsT2{b}", bufs=4)
        nc.sync.dma_start(xsT2t[:, :], P["xsT2"][b, :, :])
        d["xsT2"] = xsT2t
        stT = dat.tile([DOUT, N], BF16, tag="stT", name=f"stT{b}", bufs=5)
        nc.sync.dma_start(stT[:, :], P["stateT"][b, :, :])
        d["stT"] = stT
        xTb = act.tile([DIN, N], BF16, tag="xTb", name=f"xTb{b}", bufs=4)
        nc.vector.tensor_copy(xTb[:, :], xsTt[0:DIN, :])
        d["xTb"] = xTb
        tdso = []
        for nm in ("tT", "dT", "sT", "oT"):
            t = dat.tile([E, N], BF16, tag=nm, name=f"{nm}{b}", bufs=2)
            nc.sync.dma_start(t[:, :], P[nm][b, :, :])
            tdso.append(t)
        p1 = act.tile([E, N], BF16, tag="p1", name=f"p1_{b}", bufs=2)
        nc.vector.tensor_mul(p1[:, :], tdso[0][:, :], tdso[1][:, :])
        p2 = act.tile([E, N], BF16, tag="p2", name=f"p2_{b}", bufs=1)
        nc.vector.tensor_mul(p2[:, :], tdso[2][:, :], tdso[3][:, :])
        p3 = act.tile([E, N], BF16, tag="p1", name=f"p3_{b}", bufs=2)
        nc.vector.tensor_mul(p3[:, :], p1[:, :], p2[:, :])
        Mb = act.tile([E, N], BF16, tag="Mb", name=f"Mb{b}", bufs=5)
        nc.vector.tensor_mul(Mb[:, :], p3[:, :], embT[:, :])
        d["Mb"] = Mb
        BAT[b] = d

    def P1(inst):
        """Hypernet + V. For gate instances, also triggers the batch load."""
        br, b = inst
        if br == "g":
            batch_load(b)
            st = ST[inst] = {}
            st["x0"] = BAT[b]["xsT"]
        else:
            st = ST[inst]  # created by glue(gate): has x0=candT, r_sb, cn
        xg2 = st["x0"]
        h1p = ps_pair(psa, f"h1p{br}{b}", 16, "psB")
        h1 = act.tile([16, N], BF16, tag="h1", name=f"h1{br}{b}")
        for i, (s0, sl) in enumerate(SPLITS):
            mm = nc.tensor.matmul(h1p[i][:16, :sl], fc[("w1", br)][:, :],
                                  xg2[:, s0 : s0 + sl], start=True, stop=True)
            if i:
                _noload(mm)
            nc.scalar.activation(h1[:, s0 : s0 + sl], h1p[i][:16, :sl],
                                 AF.Sigmoid, bias=fc[("b1", br)][:, :])
        h2p = ps_pair(psa, f"h2p{br}{b}", 2, "psB")
        h2 = act.tile([2, N], BF16, tag="h2", name=f"h2{br}{b}")
        for i, (s0, sl) in enumerate(SPLITS):
            mm = nc.tensor.matmul(h2p[i][:2, :sl], fc[("w2", br)][:, :],
                                  h1[:, s0 : s0 + sl], start=True, stop=True)
            if i:
                _noload(mm)
            nc.scalar.activation(h2[:, s0 : s0 + sl], h2p[i][:2, :sl],
                                 AF.Sigmoid, bias=fc[("b2", br)][:, :])
        h3p = ps_pair(psa, f"h3p{br}{b}", E, "psB")
        filt = act.tile([E, N], BF16, tag="filt", name=f"filt{br}{b}")
        for i, (s0, sl) in enumerate(SPLITS):
            mm = nc.tensor.matmul(h3p[i][:E, :sl], fc[("w3", br)][:, :],
                                  h2[:, s0 : s0 + sl], start=True, stop=True)
            if i:
                _noload(mm)
            nc.scalar.activation(filt[:, s0 : s0 + sl], h3p[i][:E, :sl],
                                 AF.Identity, bias=fc[("b3", br)][:, :])
        vpre = act.tile([E, N], BF16, tag="vpre", name=f"vpre{br}{b}")
        nc.vector.tensor_mul(vpre[:, :], BAT[b]["Mb"][:, :], filt[:, :])
        V = act.tile([E, N], BF16, tag="V", name=f"V{br}{b}")
        nc.scalar.activation(V[:, :], vpre[:, :], AF.Tanh)
        st["V"] = V
        rs0 = dnp.tile([128, 8], F32, tag="rs0", name=f"rs0{br}{b}")
        rs1 = dnp.tile([128, 8], F32, tag="rs1", name=f"rs1{br}{b}")
        nc.vector.memset(rs0[:, :], 0.5)
        nc.vector.memset(rs1[:, :], 0.5)
        st["rs"] = (rs0, rs1)

    def P2(inst):
        """A = relu(V V^T) + fused row-sums."""
        br, b = inst
        st = ST[inst]
        V, rsh = st["V"], st["rs"]
        ar = []
        for kt in range(NT):
            p = _pt(kt)
            aps = [psa.tile([128, sl], F32, tag="psB", name=f"aps{br}{b}{kt}_{i}")
                   for i, (s0, sl) in enumerate(SPLITS)]
            art = arp.tile([128, N], BF16, tag="ar", name=f"ar{br}{b}{kt}")
            for i, (s0, sl) in enumerate(SPLITS):
                mm = nc.tensor.matmul(aps[i][:p, :sl],
                                      V[:, kt * 128 : kt * 128 + p],
                                      V[:, s0 : s0 + sl], start=True, stop=True)
                if i:
                    _noload(mm)
                nc.scalar.activation(art[:p, s0 : s0 + sl], aps[i][:p, :sl],
                                     AF.Relu, accum_out=rsh[i][:p, kt : kt + 1])
            ar.append(art)
        st["ar"] = ar

    def P3(inst):
        """d = rsqrt(rowsums) on DVE; x' = d*xs."""
        br, b = inst
        st = ST[inst]
        rs0, rs1 = st["rs"]
        rsall = dnp.tile([128, 8], F32, tag="rsall", name=f"rsall{br}{b}")
        nc.vector.tensor_add(rsall[:, :], rs0[:, :], rs1[:, :])
        tsh = dnp.tile([128, 8], F32, tag="tsh", name=f"tsh{br}{b}")
        nc.vector.tensor_scalar(tsh[:, :].bitcast(I32), rsall[:, :].bitcast(I32),
                                1, None, AluOpType.logical_shift_right)
        tnot = dnp.tile([128, 8], F32, tag="tnot", name=f"tnot{br}{b}")
        nc.vector.tensor_scalar(tnot[:, :].bitcast(I32), tsh[:, :].bitcast(I32),
                                -1, None, AluOpType.bitwise_xor)
        d0 = dnp.tile([128, 8], F32, tag="d0", name=f"d0{br}{b}")
        nc.vector.tensor_scalar(d0[:, :].bitcast(I32), tnot[:, :].bitcast(I32),
                                RSQRT_MAGIC + 1, None, AluOpType.add)
        sq = dnp.tile([128, 8], F32, tag="sq", name=f"sq{br}{b}")
        nc.vector.tensor_mul(sq[:, :], d0[:, :], d0[:, :])
        hx = dnp.tile([128, 8], F32, tag="hx", name=f"hx{br}{b}")
        nc.vector.tensor_mul(hx[:, :], sq[:, :], rsall[:, :])
        cf = dnp.tile([128, 8], F32, tag="cf", name=f"cf{br}{b}")
        nc.vector.tensor_scalar(cf[:, :], hx[:, :], -0.5, 1.5,
                                AluOpType.mult, AluOpType.add)
        dcat = dnp.tile([128, 8], F32, tag="dcat", name=f"dcat{br}{b}")
        nc.vector.tensor_mul(dcat[:, :], d0[:, :], cf[:, :])
        st["dcat"] = dcat
        xnat = BAT[b]["xs_nat"] if br == "g" else st["cn"]
        xp = []
        for kt in range(NT):
            p = _pt(kt)
            xpt = xpp.tile([128, C], BF16, tag="xp", name=f"xp{br}{b}{kt}")
            nc.vector.tensor_scalar_mul(xpt[:p, :], xnat[kt][:p, :],
                                        dcat[:p, kt : kt + 1])
            xp.append(xpt)
        st["xp"] = xp

    def P4(inst):
        """d-row assembly + GPSIMD partition broadcast + y^T matmuls."""
        br, b = inst
        st = ST[inst]
        tp = psa.tile([128, 128], F32, tag="psB", name=f"dtp{br}{b}")
        nc.tensor.transpose(tp[:8, :128], st["dcat"][:, :], ident_f[:, :])
        drs = act.tile([8, 128], BF16, tag="drs", name=f"drs{br}{b}")
        nc.scalar.copy(drs[:, :], tp[:8, :128])
        drow = act.tile([1, N], BF16, tag="drow", name=f"drow{br}{b}")
        for k in range(6):
            nc.sync.dma_start(drow[0:1, k * 128 : (k + 1) * 128], drs[k : k + 1, :])
        nc.sync.dma_start(drow[0:1, 768:N], drs[6:7, 0 : N - 768])
        dB2 = act.tile([128, N], BF16, tag="dB2", name=f"dB2{br}{b}")
        nc.gpsimd.partition_broadcast(dB2[:, :], drow[0:1, :])
        st["dB2"] = dB2
        yt = ps_pair(psp, f"yt{br}{b}", C, "psA")
        ar, xp = st["ar"], st["xp"]
        for kt in range(NT):
            p = _pt(kt)
            for i, (s0, sl) in enumerate(SPLITS):
                mm = nc.tensor.matmul(yt[i][:C, :sl], xp[kt][:p, :],
                                      ar[kt][:p, s0 : s0 + sl],
                                      start=(kt == 0), stop=(kt == NT - 1))
                if i:
                    _noload(mm)
        st["yt"] = yt

    def P5(inst):
        """Lx^T = x^T - dB2*y^T; state rows doubled to lx2 via DMA; x-rows
        of Lx and of x packed into xq (4, N) for the P6 x-part."""
        br, b = inst
        st = ST[inst]
        yt, dB2 = st["yt"], st["dB2"]
        yd = act.tile([C, N], BF16, tag="yd", name=f"yd{br}{b}")
        for i, (s0, sl) in enumerate(SPLITS):
            nc.vector.tensor_mul(yd[:, s0 : s0 + sl], yt[i][:C, :sl],
                                 dB2[0:C, s0 : s0 + sl])
        lxT = act.tile([C, N], BF16, tag="lxT", name=f"lxT{br}{b}", bufs=4)
        nc.vector.tensor_sub(lxT[:, :], st["x0"][:, :], yd[:, :])
        lx2 = act.tile([128, N], BF16, tag="lx2", name=f"lx2{br}{b}", bufs=4)
        nc.sync.dma_start(lx2[0:DOUT, :], lxT[DIN:C, :])
        nc.sync.dma_start(lx2[DOUT:128, :], lxT[DIN:C, :])
        st["lx2"] = lx2
        # xq rows: [Lx row0, Lx row1, x row0, x row1]
        xq = act.tile([2 * DIN, N], BF16, tag="xq", name=f"xq{br}{b}", bufs=2)
        nc.vector.tensor_sub(xq[0:DIN, :], st["x0"][0:DIN, :], yd[0:DIN, :])
        nc.sync.dma_start(xq[DIN : 2 * DIN, :], st["x0"][0:DIN, :])
        st["xq"] = xq

    def P6(inst):
        """Final per-node einsum (packed 128-row contraction) + activation.

        out^T = bpool^T embT + sum_{k,ep} wz64[k*5+ep]^T (embP[ep] * sg2_k)
                + wz40^T (embX * xrep),  sg2_0/1 = state rows doubled."""
        br, b = inst
        st = ST[inst]
        On = OG if br == "g" else OU
        outf = AF.Sigmoid if br == "g" else AF.Tanh
        x02 = BAT[b]["xsT2"] if br == "g" else st["cand2"]
        # x-rows replicated to the j*10+e pattern via a K=4 selection matmul
        NR = 2 * CHEB * E  # 40
        xrp = ps_pair(psa, f"xrp{br}{b}", NR, "psB")
        zx = act.tile([NR, N], BF16, tag="zx", name=f"zx{br}{b}")
        for i, (s0, sl) in enumerate(SPLITS):
            mm = nc.tensor.matmul(xrp[i][:NR, :sl], sel4[:, :],
                                  st["xq"][:, s0 : s0 + sl],
                                  start=True, stop=True)
            if i:
                _noload(mm)
            nc.vector.tensor_mul(zx[:, s0 : s0 + sl], embX[:, s0 : s0 + sl],
                                 xrp[i][:NR, :sl])
        op = ps_pair(psp, f"op{br}{b}", On, "psA")
        for i, (s0, sl) in enumerate(SPLITS):
            mm = nc.tensor.matmul(op[i][:On, :sl], bp[br][:, :],
                                  embT[:, s0 : s0 + sl], start=True, stop=False)
            if i:
                _noload(mm)
        for k in range(CHEB):
            src = x02 if k == 0 else st["lx2"]
            for ep in range(5):
                zb = ztp.tile([128, N], BF16, tag="zt", name=f"zb{br}{b}{k}{ep}")
                # k=0 zb inputs are ready slots earlier; offload some to the
                # idle GPSIMD engine to relieve DVE
                eng = nc.gpsimd if (k == 0 and ep < 3) else nc.vector
                eng.tensor_mul(zb[:, :], embP[ep][:, :], src[:, :])
                for i, (s0, sl) in enumerate(SPLITS):
                    mm = nc.tensor.matmul(op[i][:On, :sl],
                                          wz64[br][k * 5 + ep][:, :],
                                          zb[:, s0 : s0 + sl], start=False,
                                          stop=False)
                    if i:
                        _noload(mm)
        for i, (s0, sl) in enumerate(SPLITS):
            mm = nc.tensor.matmul(op[i][:On, :sl], wz40[br][:, :],
                                  zx[:, s0 : s0 + sl], start=False, stop=True)
            if i:
                _noload(mm)
        zout = act.tile([On, N], BF16, tag=f"zout{br}", name=f"zout{br}{b}")
        for i, (s0, sl) in enumerate(SPLITS):
            nc.scalar.activation(zout[:, s0 : s0 + sl], op[i][:On, :sl], outf)
        st["zout"] = zout

    def glue(inst):
        """After P6: gate -> build update inputs; update -> epilogue + store."""
        br, b = inst
        if br == "g":
            zr = ST[inst]["zout"]  # (128, N): z rows 0:64, r rows 64:128
            stT = BAT[b]["stT"]
            ust = {}
            ST[("u", b)] = ust
            cand2 = act.tile([128, N], BF16, tag="cand2", name=f"cand2{b}",
                             bufs=4)
            nc.vector.tensor_mul(cand2[0:DOUT, :], zr[0:DOUT, :], stT[:, :])
            nc.sync.dma_start(cand2[DOUT:128, :], cand2[0:DOUT, :])
            ust["cand2"] = cand2
            candT = act.tile([C, N], BF16, tag="candT", name=f"candT{b}", bufs=4)
            nc.sync.dma_start(candT[0:DIN, :], BAT[b]["xTb"][:, :])
            nc.sync.dma_start(candT[DIN:C, :], cand2[0:DOUT, :])
            ust["x0"] = candT
            r_sb = act.tile([DOUT, N], BF16, tag="r_sb", name=f"r_sb{b}", bufs=4)
            nc.sync.dma_start(r_sb[:, :], zr[DOUT:OG, :])
            ust["r_sb"] = r_sb
            cn_l = []
            for nt in range(NT):
                p = _pt(nt)
                zps = psa.tile([128, 128], BF16, tag="psB", name=f"znp{b}{nt}")
                nc.tensor.transpose(zps[:p, :DOUT],
                                    zr[0:DOUT, nt * 128 : nt * 128 + p],
                                    ident_b[:DOUT, :DOUT])
                zn = act.tile([128, DOUT], BF16, tag="zn", name=f"zn{b}{nt}",
                              bufs=4)
                nc.scalar.copy(zn[:p, :], zps[:p, :DOUT])
                cn = xnp.tile([128, C], BF16, tag="cn", name=f"cn{b}{nt}", bufs=22)
                nc.vector.tensor_copy(cn[:p, 0:DIN],
                                      BAT[b]["xs_nat"][nt][:p, 0:DIN])
                nc.vector.tensor_mul(cn[:p, DIN:C], zn[:p, :],
                                     BAT[b]["xs_nat"][nt][:p, DIN:C])
                cn_l.append(cn)
            ust["cn"] = cn_l
        else:
            hc = ST[inst]["zout"]  # (64, N)
            stT = BAT[b]["stT"]
            r_sb = ST[inst]["r_sb"]
            t1 = act.tile([OU, N], BF16, tag="t1", name=f"t1_{b}", bufs=2)
            nc.vector.tensor_sub(t1[:, :], stT[:, :], hc[:, :])
            t2 = act.tile([OU, N], BF16, tag="t2", name=f"t2_{b}", bufs=2)
            nc.vector.tensor_mul(t2[:, :], r_sb[:, :], t1[:, :])
            outT = act.tile([OU, N], BF16, tag="outT", name=f"outT{b}")
            nc.vector.tensor_add(outT[:, :], t2[:, :], hc[:, :])
            nc.sync.dma_start(P["out"][b, :, :], outT[:, :])

    # ---------------- pipeline driver ----------------
    M = len(SEQ)
    P1(SEQ[0])
    for s in range(M + 2):
        if 0 <= s - 2 < M:
            P6(SEQ[s - 2])
            glue(SEQ[s - 2])
        if 0 <= s - 1 < M:
            P4(SEQ[s - 1])
            P5(SEQ[s - 1])
        if s < M:
            P2(SEQ[s])
            P3(SEQ[s])
        if s + 1 < M:
            P1(SEQ[s + 1])


def build_nc():
    nc = bacc.Bacc()
    P = {}

    def dp(name, shape, dtype=F32, out=False):
        P[name] = nc.declare_dram_parameter(name, list(shape), dtype, isOutput=out)

    dp("x", (BL, N, DIN), BF16)
    dp("state", (BL, N, DOUT), BF16)
    dp("xsT", (BL, C, N), BF16)
    dp("xsT2", (BL, 128, N), BF16)
    dp("stateT", (BL, DOUT, N), BF16)
    for nm in ("tT", "dT", "sT", "oT"):
        dp(nm, (BL, E, N), BF16)
    dp("embT", (E, N), BF16)
    dp("embP", (5 * 128, N), BF16)
    dp("embX", (2 * CHEB * E, N), BF16)
    dp("wz64_g", (10 * 128, OG), BF16)
    dp("wz64_u", (10 * 128, OU), BF16)
    dp("wz40_g", (2 * CHEB * E, OG), BF16)
    dp("wz40_u", (2 * CHEB * E, OU), BF16)
    dp("sel4", (2 * DIN, 2 * CHEB * E), BF16)
    dp("bpool_g", (E, OG), BF16)
    dp("bpool_u", (E, OU), BF16)
    for br in ("g", "u"):
        dp(f"fc1w_{br}", (C, 16), BF16)
        dp(f"fc2w_{br}", (16, 2), BF16)
        dp(f"fc3w_{br}", (2, E), BF16)
        dp(f"fcb1_{br}", (16, 1))
        dp(f"fcb2_{br}", (2, 1))
        dp(f"fcb3_{br}", (E, 1))
    dp("ident", (128, 128))
    dp("out", (BL, OU, N), BF16, out=True)
    with tile.TileContext(nc) as tc:
        with ExitStack() as ctx:
            _build_body(tc, ctx, nc, P)
    nc.finalize()
    return nc


_NC_CACHE = {}


def _get_nc():
    if "nc" not in _NC_CACHE:
        _NC_CACHE["nc"] = build_nc()
    return _NC_CACHE["nc"]


def _make_in_maps(inputs):
    f32 = lambda a: np.ascontiguousarray(a, dtype=np.float32)
    bf = lambda a: np.ascontiguousarray(np.asarray(a, dtype=np.float32).astype(BF16_NP))
    x = f32(inputs["x"])
    state = f32(inputs["state"])
    emb = f32(inputs["node_embeddings"])
    time, day = f32(inputs["time"]), f32(inputs["day"])
    speed, occupy = f32(inputs["speed"]), f32(inputs["occupy"])
    xs = np.concatenate([x, state], axis=-1)

    def wz64_prep(wpool):
        # tile t = k*5+ep: rows [w[2ep,k,2:66,:]; w[2ep+1,k,2:66,:]]
        return np.concatenate(
            [wpool[2 * ep + h, k, DIN:C, :]
             for k in range(CHEB) for ep in range(5) for h in (0, 1)],
            axis=0)

    def wz40_prep(wpool):
        # row j*10+e = w[e,k,c,:] with j-order [(1,0),(1,1),(0,0),(0,1)]
        # (matches xq rows [Lx0, Lx1, x0, x1])
        return np.concatenate(
            [wpool[:, k, c, :] for (k, c) in ((1, 0), (1, 1), (0, 0), (0, 1))],
            axis=0)

    shared = {
        "embT": bf(emb.T),
        "embP": bf(np.repeat(emb.T, DOUT, axis=0)),
        "embX": bf(np.tile(emb.T, (2 * CHEB, 1))),
        "wz64_g": bf(wz64_prep(inputs["gate_wpool"])),
        "wz64_u": bf(wz64_prep(inputs["update_wpool"])),
        "wz40_g": bf(wz40_prep(inputs["gate_wpool"])),
        "wz40_u": bf(wz40_prep(inputs["update_wpool"])),
        "sel4": bf(np.repeat(np.eye(2 * DIN, dtype=np.float32), E, axis=1)),
        "bpool_g": bf(inputs["gate_bpool"]),
        "bpool_u": bf(inputs["update_bpool"]),
        "ident": np.eye(128, dtype=np.float32),
    }
    for br, pre in (("g", "gate"), ("u", "update")):
        shared[f"fc1w_{br}"] = bf(inputs[f"{pre}_fc1_w"])
        shared[f"fc2w_{br}"] = bf(inputs[f"{pre}_fc2_w"])
        shared[f"fc3w_{br}"] = bf(inputs[f"{pre}_fc3_w"])
        shared[f"fcb1_{br}"] = f32(inputs[f"{pre}_fc1_b"].reshape(16, 1))
        shared[f"fcb2_{br}"] = f32(inputs[f"{pre}_fc2_b"].reshape(2, 1))
        shared[f"fcb3_{br}"] = f32(inputs[f"{pre}_fc3_b"].reshape(E, 1))

    in_maps = []
    for c in range(NCORES):
        sl = slice(c * BL, (c + 1) * BL)
        m = dict(shared)
        m["x"] = bf(x[sl])
        m["state"] = bf(state[sl])
        m["xsT"] = bf(xs[sl].transpose(0, 2, 1))
        xsT_c = xs[sl].transpose(0, 2, 1)
        m["xsT2"] = bf(np.concatenate([xsT_c[:, DIN:C, :]] * 2, axis=1))
        m["stateT"] = bf(state[sl].transpose(0, 2, 1))
        m["tT"] = bf(time[sl].transpose(0, 2, 1))
        m["dT"] = bf(day[sl].transpose(0, 2, 1))
        m["sT"] = bf(speed[sl].transpose(0, 2, 1))
        m["oT"] = bf(occupy[sl].transpose(0, 2, 1))
        in_maps.append(m)
    return in_maps


def _run(inputs, trace=False):
    nc = _get_nc()
    in_maps = _make_in_maps(inputs)
    res = run_bass_kernel_spmd(nc, in_maps, core_ids=list(range(NCORES)), trace=trace)
    out = np.concatenate(
        [np.asarray(res.results[i]["out"]).transpose(0, 2, 1) for i in range(NCORES)],
        axis=0,
    )
    return out.astype(np.float32), res


def kernel(**inputs):
    out, _ = _run(inputs, trace=False)
    return out



# revision 41
# speedup vs baseline: 1.4244x; 1.4244x over previous
"""DDGCRN cell on 8 TRN2 NeuronCores — data-parallel over batch.

Per core: 8 batches = 16 branch-instances (gate O=128 / update O=64), emitted
as a software pipeline so every engine's static instruction stream stays
dense (engines execute their streams in order; serial per-instance chains
would otherwise stall the TensorEngine and re-throttle its HAM clock gate).

Pipeline: step s emits  P6(s-2) op-matmuls+activation | P4(s-1) d-bcast+yT
| P5(s-1) Lx | P2(s) A-matmuls+relu+rowsum | P3(s) rsqrt+x' | P1(s+1)
hypernet+V.  Update(b) is sequenced >=3 slots after gate(b) (needs z).

Math per instance:
  filt = hypernet MLP (transposed-feature layout, bf16)
  V = tanh(emb*time*day*speed*occupy*filt)      (10, 883)
  A = relu(V V^T) (883,883 symmetric) + fused row-sums (ACT accum_out)
  d = rsqrt(rowsum) via fast-inverse-sqrt + 1 Newton step (DVE only; keeps
      ScalarE pinned to the sigmoid/tanh/relu table set — no table reloads)
  Lx^T = xs^T - dB2 * ((d*xs)^T A)  using A's symmetry; dB2 = GPSIMD
      partition-broadcast of the d row (128 rows: state part doubled)
  out^T = bpool.T @ embT + sum_{k,ep} wz64.T @ (embP_ep * sg2_k)
        + wz40.T @ (embX * xrep)
      (state rows packed in 64-row pairs -> 10 K=128 matmuls; the 4 x-rows
       replicated per-e via GPSIMD broadcast -> 1 K=40 matmul)

All matmuls bf16 (PSUM f32); inputs pre-cast/pre-transposed on host (pure
layout/dtype prep). Output written transposed, un-transposed on host.
"""

import sys, os

sys.path.insert(0, "/opt/trn_rl_repo")

import numpy as np
import ml_dtypes
from contextlib import ExitStack

import concourse.bass as bass
import concourse.bacc as bacc
import concourse.mybir as mybir
from concourse import tile
from concourse.alu_op_type import AluOpType
from concourse.bass_utils import run_bass_kernel_spmd

AF = mybir.ActivationFunctionType
F32 = mybir.dt.float32
BF16 = mybir.dt.bfloat16
I32 = mybir.dt.int32
BF16_NP = ml_dtypes.bfloat16

B, N, DIN, DOUT, E, CHEB = 64, 883, 2, 64, 10, 2
C = DIN + DOUT  # 66
NCORES = 8
BL = B // NCORES  # 8 batches per core
NT = (N + 127) // 128  # 7 row tiles
EKC = E * CHEB * C  # 1320
OG, OU = 2 * DOUT, DOUT  # 128, 64
SPLITS = [(0, 512), (512, N - 512)]
RSQRT_MAGIC = 0x5F3759DF

# instance schedule: update(b) >= 3 slots after gate(b)
SEQ = [("g", 0), ("g", 1), ("g", 2), ("u", 0), ("g", 3), ("u", 1), ("g", 4),
       ("u", 2), ("g", 5), ("u", 3), ("g", 6), ("u", 4), ("g", 7), ("u", 5),
       ("u", 6), ("u", 7)]


def _pt(nt):
    return min(128, N - nt * 128)


def _noload(mm):
    """Mark a matmul as reusing the PE-resident weights (skips LDWEIGHTS).

    Only valid when the immediately preceding PE-queue instruction loaded
    identical weights (the first matmul of a PSUM-split pair)."""
    mm.ins.ldweights = False
    return mm


def _build_body(tc, ctx, nc, P):
    def pool(name, bufs, space="SBUF"):
        return ctx.enter_context(tc.tile_pool(name=name, bufs=bufs, space=space))

    wp = pool("wp", 1)        # static weights
    dat = pool("dat", 2)      # per-batch DMA loads
    act = pool("act", 2)      # per-instance intermediates
    arp = pool("arp", 14)     # relu(A) tiles: 2 instances x 7 in flight
    xnp = pool("xnp", 29)     # natural xs/cand tiles
    xpp = pool("xpp", 15)     # d*xs tiles
    ztp = pool("ztp", 2)      # packed zb tiles (double buffer)
    dnp = pool("dnp", 4)      # rowsum/d helpers
    psp = pool("psp", 4, space="PSUM")  # op + yT accumulators (tag psA)
    psa = pool("psa", 4, space="PSUM")  # A halves / hypernet / transposes

    def ps_pair(p, name, parts, tag):
        return [p.tile([parts, sl], F32, tag=tag, name=f"{name}_{i}")
                for i, (s0, sl) in enumerate(SPLITS)]

    # ---------------- static setup (all bf16 direct from host) ----------
    ident_f = wp.tile([128, 128], F32, tag="identf", name="ident_f")
    nc.sync.dma_start(ident_f[:, :], P["ident"][:, :])
    ident_b = wp.tile([128, 128], BF16, tag="identb", name="ident_b")
    nc.vector.tensor_copy(ident_b[:, :], ident_f[:, :])

    def load_bf(pname, shape, tag):
        t = wp.tile(list(shape), BF16, tag=tag, name=pname + "_t")
        nc.sync.dma_start(t[:, :], P[pname][:, :])
        return t

    embT = load_bf("embT", (E, N), "embT")
    # emb broadcast tiles for the packed P6 contraction: embP[ep] rows =
    # [emb[:,2ep] x64 ; emb[:,2ep+1] x64]; embX rows (2k+c)*10+e = emb[:,e]
    embP = []
    for ep in range(5):
        t = wp.tile([128, N], BF16, tag=f"embP{ep}", name=f"embP{ep}")
        nc.sync.dma_start(t[:, :], P["embP"][ep * 128 : (ep + 1) * 128, :])
        embP.append(t)
    embX = load_bf("embX", (2 * CHEB * E, N), "embX")
    sel4 = load_bf("sel4", (2 * DIN, 2 * CHEB * E), "sel4")
    wz64 = {}
    wz40 = {}
    for br, On in (("g", OG), ("u", OU)):
        tiles = []
        for t_i in range(2 * 5):
            t = wp.tile([128, On], BF16, tag=f"wz64{br}{t_i}", name=f"wz64{br}{t_i}")
            nc.sync.dma_start(t[:, :], P[f"wz64_{br}"][t_i * 128 : (t_i + 1) * 128, :])
            tiles.append(t)
        wz64[br] = tiles
        wz40[br] = load_bf(f"wz40_{br}", (2 * CHEB * E, On), f"wz40{br}")
    fc = {}
    for br in ("g", "u"):
        fc[("w1", br)] = load_bf(f"fc1w_{br}", (C, 16), f"fc1w{br}")
        fc[("w2", br)] = load_bf(f"fc2w_{br}", (16, 2), f"fc2w{br}")
        fc[("w3", br)] = load_bf(f"fc3w_{br}", (2, E), f"fc3w{br}")
        for nm, shape in (("b1", (16, 1)), ("b2", (2, 1)), ("b3", (E, 1))):
            t = wp.tile(list(shape), F32, tag=f"fc{nm}{br}", name=f"fc{nm}{br}")
            nc.sync.dma_start(t[:, :], P[f"fc{nm}_{br}"][:, :])
            fc[(nm, br)] = t
    bp = {"g": load_bf("bpool_g", (E, OG), "bpg"),
          "u": load_bf("bpool_u", (E, OU), "bpu")}

    # ---------------- per-instance state ----------------
    ST = {}   # (br,b) -> dict of tiles
    BAT = {}  # b -> dict of per-batch tiles

    def batch_load(b):
        """DMA this batch's inputs; build Mb and gate xg2."""
        d = {}
        xs_nat = []
        for nt in range(NT):
            p = _pt(nt)
            t = xnp.tile([128, C], BF16, tag="xsn", name=f"xsn{b}{nt}")
            nc.sync.dma_start(t[:p, 0:DIN], P["x"][b, nt * 128 : nt * 128 + p, :])
            nc.sync.dma_start(t[:p, DIN:C], P["state"][b, nt * 128 : nt * 128 + p, :])
            xs_nat.append(t)
        d["xs_nat"] = xs_nat
        xsTt = act.tile([C, N], BF16, tag="xsT", name=f"xsT{b}", bufs=4)
        nc.sync.dma_start(xsTt[:, :], P["xsT"][b, :, :])
        d["xsT"] = xsTt
        xsT2t = act.tile([128, N], BF16, tag="xsT2", name=f"xsT2{b}", bufs=4)
        nc.sync.dma_start(xsT2t[:, :], P["xsT2"][b, :, :])
        d["xsT2"] = xsT2t
        stT = dat.tile([DOUT, N], BF16, tag="stT", name=f"stT{b}", bufs=5)
        nc.sync.dma_start(stT[:, :], P["stateT"][b, :, :])
        d["stT"] = stT
        xTb = act.tile([DIN, N], BF16, tag="xTb", name=f"xTb{b}", bufs=4)
        nc.vector.tensor_copy(xTb[:, :], xsTt[0:DIN, :])
        d["xTb"] = xTb
        tdso = []
        for nm in ("tT", "dT", "sT", "oT"):
            t = dat.tile([E, N], BF16, tag=nm, name=f"{nm}{b}", bufs=2)
            nc.sync.dma_start(t[:, :], P[nm][b, :, :])
            tdso.append(t)
        p1 = act.tile([E, N], BF16, tag="p1", name=f"p1_{b}", bufs=2)
        nc.vector.tensor_mul(p1[:, :], tdso[0][:, :], tdso[1][:, :])
        p2 = act.tile([E, N], BF16, tag="p2", name=f"p2_{b}", bufs=1)
        nc.vector.tensor_mul(p2[:, :], tdso[2][:, :], tdso[3][:, :])
        p3 = act.tile([E, N], BF16, tag="p1", name=f"p3_{b}", bufs=2)
        nc.vector.tensor_mul(p3[:, :], p1[:, :], p2[:, :])
        Mb = act.tile([E, N], BF16, tag="Mb", name=f"Mb{b}", bufs=5)
        nc.vector.tensor_mul(Mb[:, :], p3[:, :], embT[:, :])
        d["Mb"] = Mb
        BAT[b] = d

    def P1(inst):
        """Hypernet + V. For gate instances, also triggers the batch load."""
        br, b = inst
        if br == "g":
            batch_load(b)
            st = ST[inst] = {}
            st["x0"] = BAT[b]["xsT"]
        else:
            st = ST[inst]  # created by glue(gate): has x0=candT, r_sb, cn
        xg2 = st["x0"]
        h1p = ps_pair(psa, f"h1p{br}{b}", 16, "psB")
        h1 = act.tile([16, N], BF16, tag="h1", name=f"h1{br}{b}")
        for i, (s0, sl) in enumerate(SPLITS):
            mm = nc.tensor.matmul(h1p[i][:16, :sl], fc[("w1", br)][:, :],
                                  xg2[:, s0 : s0 + sl], start=True, stop=True)
            if i:
                _noload(mm)
            nc.scalar.activation(h1[:, s0 : s0 + sl], h1p[i][:16, :sl],
                                 AF.Sigmoid, bias=fc[("b1", br)][:, :])
        h2p = ps_pair(psa, f"h2p{br}{b}", 2, "psB")
        h2 = act.tile([2, N], BF16, tag="h2", name=f"h2{br}{b}")
        for i, (s0, sl) in enumerate(SPLITS):
            mm = nc.tensor.matmul(h2p[i][:2, :sl], fc[("w2", br)][:, :],
                                  h1[:, s0 : s0 + sl], start=True, stop=True)
            if i:
                _noload(mm)
            nc.scalar.activation(h2[:, s0 : s0 + sl], h2p[i][:2, :sl],
                                 AF.Sigmoid, bias=fc[("b2", br)][:, :])
        h3p = ps_pair(psa, f"h3p{br}{b}", E, "psB")
        filt = act.tile([E, N], BF16, tag="filt", name=f"filt{br}{b}")
        for i, (s0, sl) in enumerate(SPLITS):
            mm = nc.tensor.matmul(h3p[i][:E, :sl], fc[("w3", br)][:, :],
                                  h2[:, s0 : s0 + sl], start=True, stop=True)
            if i:
                _noload(mm)
            nc.scalar.activation(filt[:, s0 : s0 + sl], h3p[i][:E, :sl],
                                 AF.Identity, bias=fc[("b3", br)][:, :])
        vpre = act.tile([E, N], BF16, tag="vpre", name=f"vpre{br}{b}")
        nc.vector.tensor_mul(vpre[:, :], BAT[b]["Mb"][:, :], filt[:, :])
        V = act.tile([E, N], BF16, tag="V", name=f"V{br}{b}")
        nc.scalar.activation(V[:, :], vpre[:, :], AF.Tanh)
        st["V"] = V
        rs0 = dnp.tile([128, 8], F32, tag="rs0", name=f"rs0{br}{b}")
        rs1 = dnp.tile([128, 8], F32, tag="rs1", name=f"rs1{br}{b}")
        nc.vector.memset(rs0[:, :], 0.5)
        nc.vector.memset(rs1[:, :], 0.5)
        st["rs"] = (rs0, rs1)

    def P2(inst):
        """A = relu(V V^T) + fused row-sums."""
        br, b = inst
        st = ST[inst]
        V, rsh = st["V"], st["rs"]
        ar = []
        for kt in range(NT):
            p = _pt(kt)
            aps = [psa.tile([128, sl], F32, tag="psB", name=f"aps{br}{b}{kt}_{i}")
                   for i, (s0, sl) in enumerate(SPLITS)]
            art = arp.tile([128, N], BF16, tag="ar", name=f"ar{br}{b}{kt}")
            for i, (s0, sl) in enumerate(SPLITS):
                mm = nc.tensor.matmul(aps[i][:p, :sl],
                                      V[:, kt * 128 : kt * 128 + p],
                                      V[:, s0 : s0 + sl], start=True, stop=True)
                if i:
                    _noload(mm)
                nc.scalar.activation(art[:p, s0 : s0 + sl], aps[i][:p, :sl],
                                     AF.Relu, accum_out=rsh[i][:p, kt : kt + 1])
            ar.append(art)
        st["ar"] = ar

    def P3(inst):
        """d = rsqrt(rowsums) on DVE; x' = d*xs."""
        br, b = inst
        st = ST[inst]
        rs0, rs1 = st["rs"]
        rsall = dnp.tile([128, 8], F32, tag="rsall", name=f"rsall{br}{b}")
        nc.vector.tensor_add(rsall[:, :], rs0[:, :], rs1[:, :])
        tsh = dnp.tile([128, 8], F32, tag="tsh", name=f"tsh{br}{b}")
        nc.vector.tensor_scalar(tsh[:, :].bitcast(I32), rsall[:, :].bitcast(I32),
                                1, None, AluOpType.logical_shift_right)
        tnot = dnp.tile([128, 8], F32, tag="tnot", name=f"tnot{br}{b}")
        nc.vector.tensor_scalar(tnot[:, :].bitcast(I32), tsh[:, :].bitcast(I32),
                                -1, None, AluOpType.bitwise_xor)
        d0 = dnp.tile([128, 8], F32, tag="d0", name=f"d0{br}{b}")
        nc.vector.tensor_scalar(d0[:, :].bitcast(I32), tnot[:, :].bitcast(I32),
                                RSQRT_MAGIC + 1, None, AluOpType.add)
        sq = dnp.tile([128, 8], F32, tag="sq", name=f"sq{br}{b}")
        nc.vector.tensor_mul(sq[:, :], d0[:, :], d0[:, :])
        hx = dnp.tile([128, 8], F32, tag="hx", name=f"hx{br}{b}")
        nc.vector.tensor_mul(hx[:, :], sq[:, :], rsall[:, :])
        cf = dnp.tile([128, 8], F32, tag="cf", name=f"cf{br}{b}")
        nc.vector.tensor_scalar(cf[:, :], hx[:, :], -0.5, 1.5,
                                AluOpType.mult, AluOpType.add)
        dcat = dnp.tile([128, 8], F32, tag="dcat", name=f"dcat{br}{b}")
        nc.vector.tensor_mul(dcat[:, :], d0[:, :], cf[:, :])
        st["dcat"] = dcat
        xnat = BAT[b]["xs_nat"] if br == "g" else st["cn"]
        xp = []
        for kt in range(NT):
            p = _pt(kt)
            xpt = xpp.tile([128, C], BF16, tag="xp", name=f"xp{br}{b}{kt}")
            nc.vector.tensor_scalar_mul(xpt[:p, :], xnat[kt][:p, :],
                                        dcat[:p, kt : kt + 1])
            xp.append(xpt)
        st["xp"] = xp

    def P4(inst):
        """d-row assembly + GPSIMD partition broadcast + y^T matmuls."""
        br, b = inst
        st = ST[inst]
        tp = psa.tile([128, 128], F32, tag="psB", name=f"dtp{br}{b}")
        nc.tensor.transpose(tp[:8, :128], st["dcat"][:, :], ident_f[:, :])
        drs = act.tile([8, 128], BF16, tag="drs", name=f"drs{br}{b}")
        nc.scalar.copy(drs[:, :], tp[:8, :128])
        drow = act.tile([1, N], BF16, tag="drow", name=f"drow{br}{b}")
        for k in range(6):
            nc.sync.dma_start(drow[0:1, k * 128 : (k + 1) * 128], drs[k : k + 1, :])
        nc.sync.dma_start(drow[0:1, 768:N], drs[6:7, 0 : N - 768])
        dB2 = act.tile([128, N], BF16, tag="dB2", name=f"dB2{br}{b}")
        nc.gpsimd.partition_broadcast(dB2[:, :], drow[0:1, :])
        st["dB2"] = dB2
        yt = ps_pair(psp, f"yt{br}{b}", C, "psA")
        ar, xp = st["ar"], st["xp"]
        for kt in range(NT):
            p = _pt(kt)
            for i, (s0, sl) in enumerate(SPLITS):
                mm = nc.tensor.matmul(yt[i][:C, :sl], xp[kt][:p, :],
                                      ar[kt][:p, s0 : s0 + sl],
                                      start=(kt == 0), stop=(kt == NT - 1))
                if i:
                    _noload(mm)
        st["yt"] = yt

    def P5(inst):
        """Lx^T = x^T - dB2*y^T; state rows doubled to lx2 via DMA; x-rows
        of Lx and of x packed into xq (4, N) for the P6 x-part."""
        br, b = inst
        st = ST[inst]
        yt, dB2 = st["yt"], st["dB2"]
        yd = act.tile([C, N], BF16, tag="yd", name=f"yd{br}{b}")
        for i, (s0, sl) in enumerate(SPLITS):
            nc.vector.tensor_mul(yd[:, s0 : s0 + sl], yt[i][:C, :sl],
                                 dB2[0:C, s0 : s0 + sl])
        lxT = act.tile([C, N], BF16, tag="lxT", name=f"lxT{br}{b}", bufs=4)
        nc.vector.tensor_sub(lxT[:, :], st["x0"][:, :], yd[:, :])
        lx2 = act.tile([128, N], BF16, tag="lx2", name=f"lx2{br}{b}", bufs=4)
        nc.sync.dma_start(lx2[0:DOUT, :], lxT[DIN:C, :])
        nc.sync.dma_start(lx2[DOUT:128, :], lxT[DIN:C, :])
        st["lx2"] = lx2
        # xq rows: [Lx row0, Lx row1, x row0, x row1]
        xq = act.tile([2 * DIN, N], BF16, tag="xq", name=f"xq{br}{b}", bufs=2)
        nc.vector.tensor_sub(xq[0:DIN, :], st["x0"][0:DIN, :], yd[0:DIN, :])
        nc.sync.dma_start(xq[DIN : 2 * DIN, :], st["x0"][0:DIN, :])
        st["xq"] = xq

    def P6(inst):
        """Final per-node einsum (packed 128-row contraction) + activation.

        out^T = bpool^T embT + sum_{k,ep} wz64[k*5+ep]^T (embP[ep] * sg2_k)
                + wz40^T (embX * xrep),  sg2_0/1 = state rows doubled."""
        br, b = inst
        st = ST[inst]
        On = OG if br == "g" else OU
        outf = AF.Sigmoid if br == "g" else AF.Tanh
        x02 = BAT[b]["xsT2"] if br == "g" else st["cand2"]
        # x-rows replicated to the j*10+e pattern via a K=4 selection matmul
        NR = 2 * CHEB * E  # 40
        xrp = ps_pair(psa, f"xrp{br}{b}", NR, "psB")
        zx = act.tile([NR, N], BF16, tag="zx", name=f"zx{br}{b}")
        for i, (s0, sl) in enumerate(SPLITS):
            mm = nc.tensor.matmul(xrp[i][:NR, :sl], sel4[:, :],
                                  st["xq"][:, s0 : s0 + sl],
                                  start=True, stop=True)
            if i:
                _noload(mm)
            nc.vector.tensor_mul(zx[:, s0 : s0 + sl], embX[:, s0 : s0 + sl],
                                 xrp[i][:NR, :sl])
        op = ps_pair(psp, f"op{br}{b}", On, "psA")
        for i, (s0, sl) in enumerate(SPLITS):
            mm = nc.tensor.matmul(op[i][:On, :sl], bp[br][:, :],
                                  embT[:, s0 : s0 + sl], start=True, stop=False)
            if i:
                _noload(mm)
        for k in range(CHEB):
            src = x02 if k == 0 else st["lx2"]
            for ep in range(5):
                zb = ztp.tile([128, N], BF16, tag="zt", name=f"zb{br}{b}{k}{ep}")
                nc.vector.tensor_mul(zb[:, :], embP[ep][:, :], src[:, :])
                for i, (s0, sl) in enumerate(SPLITS):
                    mm = nc.tensor.matmul(op[i][:On, :sl],
                                          wz64[br][k * 5 + ep][:, :],
                                          zb[:, s0 : s0 + sl], start=False,
                                          stop=False)
                    if i:
                        _noload(mm)
        for i, (s0, sl) in enumerate(SPLITS):
            mm = nc.tensor.matmul(op[i][:On, :sl], wz40[br][:, :],
                                  zx[:, s0 : s0 + sl], start=False, stop=True)
            if i:
                _noload(mm)
        zout = act.tile([On, N], BF16, tag=f"zout{br}", name=f"zout{br}{b}")
        for i, (s0, sl) in enumerate(SPLITS):
            nc.scalar.activation(zout[:, s0 : s0 + sl], op[i][:On, :sl], outf)
        st["zout"] = zout

    def glue(inst):
        """After P6: gate -> build update inputs; update -> epilogue + store."""
        br, b = inst
        if br == "g":
            zr = ST[inst]["zout"]  # (128, N): z rows 0:64, r rows 64:128
            stT = BAT[b]["stT"]
            ust = {}
            ST[("u", b)] = ust
            cand2 = act.tile([128, N], BF16, tag="cand2", name=f"cand2{b}",
                             bufs=4)
            nc.vector.tensor_mul(cand2[0:DOUT, :], zr[0:DOUT, :], stT[:, :])
            nc.sync.dma_start(cand2[DOUT:128, :], cand2[0:DOUT, :])
            ust["cand2"] = cand2
            candT = act.tile([C, N], BF16, tag="candT", name=f"candT{b}", bufs=4)
            nc.sync.dma_start(candT[0:DIN, :], BAT[b]["xTb"][:, :])
            nc.sync.dma_start(candT[DIN:C, :], cand2[0:DOUT, :])
            ust["x0"] = candT
            r_sb = act.tile([DOUT, N], BF16, tag="r_sb", name=f"r_sb{b}", bufs=4)
            nc.sync.dma_start(r_sb[:, :], zr[DOUT:OG, :])
            ust["r_sb"] = r_sb
            cn_l = []
            for nt in range(NT):
                p = _pt(nt)
                zps = psa.tile([128, 128], BF16, tag="psB", name=f"znp{b}{nt}")
                nc.tensor.transpose(zps[:p, :DOUT],
                                    zr[0:DOUT, nt * 128 : nt * 128 + p],
                                    ident_b[:DOUT, :DOUT])
                zn = act.tile([128, DOUT], BF16, tag="zn", name=f"zn{b}{nt}",
                              bufs=4)
                nc.scalar.copy(zn[:p, :], zps[:p, :DOUT])
                cn = xnp.tile([128, C], BF16, tag="cn", name=f"cn{b}{nt}", bufs=22)
                nc.vector.tensor_copy(cn[:p, 0:DIN],
                                      BAT[b]["xs_nat"][nt][:p, 0:DIN])
                nc.vector.tensor_mul(cn[:p, DIN:C], zn[:p, :],
                                     BAT[b]["xs_nat"][nt][:p, DIN:C])
                cn_l.append(cn)
            ust["cn"] = cn_l
        else:
            hc = ST[inst]["zout"]  # (64, N)
            stT = BAT[b]["stT"]
            r_sb = ST[inst]["r_sb"]
            t1 = act.tile([OU, N], BF16, tag="t1", name=f"t1_{b}", bufs=2)
            nc.vector.tensor_sub(t1[:, :], stT[:, :], hc[:, :])
            t2 = act.tile([OU, N], BF16, tag="t2", name=f"t2_{b}", bufs=2)
            nc.vector.tensor_mul(t2[:, :], r_sb[:, :], t1[:, :])
            outT = act.tile([OU, N], BF16, tag="outT", name=f"outT{b}")
            nc.vector.tensor_add(outT[:, :], t2[:, :], hc[:, :])
            nc.sync.dma_start(P["out"][b, :, :], outT[:, :])

    # ---------------- pipeline driver ----------------
    M = len(SEQ)
    P1(SEQ[0])
    for s in range(M + 2):
        if 0 <= s - 2 < M:
            P6(SEQ[s - 2])
            glue(SEQ[s - 2])
        if 0 <= s - 1 < M:
            P4(SEQ[s - 1])
            P5(SEQ[s - 1])
        if s < M:
            P2(SEQ[s])
            P3(SEQ[s])
        if s + 1 < M:
            P1(SEQ[s + 1])


def build_nc():
    nc = bacc.Bacc()
    P = {}

    def dp(name, shape, dtype=F32, out=False):
        P[name] = nc.declare_dram_parameter(name, list(shape), dtype, isOutput=out)

    dp("x", (BL, N, DIN), BF16)
    dp("state", (BL, N, DOUT), BF16)
    dp("xsT", (BL, C, N), BF16)
    dp("xsT2", (BL, 128, N), BF16)
    dp("stateT", (BL, DOUT, N), BF16)
    for nm in ("tT", "dT", "sT", "oT"):
        dp(nm, (BL, E, N), BF16)
    dp("embT", (E, N), BF16)
    dp("embP", (5 * 128, N), BF16)
    dp("embX", (2 * CHEB * E, N), BF16)
    dp("wz64_g", (10 * 128, OG), BF16)
    dp("wz64_u", (10 * 128, OU), BF16)
    dp("wz40_g", (2 * CHEB * E, OG), BF16)
    dp("wz40_u", (2 * CHEB * E, OU), BF16)
    dp("sel4", (2 * DIN, 2 * CHEB * E), BF16)
    dp("bpool_g", (E, OG), BF16)
    dp("bpool_u", (E, OU), BF16)
    for br in ("g", "u"):
        dp(f"fc1w_{br}", (C, 16), BF16)
        dp(f"fc2w_{br}", (16, 2), BF16)
        dp(f"fc3w_{br}", (2, E), BF16)
        dp(f"fcb1_{br}", (16, 1))
        dp(f"fcb2_{br}", (2, 1))
        dp(f"fcb3_{br}", (E, 1))
    dp("ident", (128, 128))
    dp("out", (BL, OU, N), BF16, out=True)
    with tile.TileContext(nc) as tc:
        with ExitStack() as ctx:
            _build_body(tc, ctx, nc, P)
    nc.finalize()
    return nc


_NC_CACHE = {}


def _get_nc():
    if "nc" not in _NC_CACHE:
        _NC_CACHE["nc"] = build_nc()
    return _NC_CACHE["nc"]


def _make_in_maps(inputs):
    f32 = lambda a: np.ascontiguousarray(a, dtype=np.float32)
    bf = lambda a: np.ascontiguousarray(np.asarray(a, dtype=np.float32).astype(BF16_NP))
    x = f32(inputs["x"])
    state = f32(inputs["state"])
    emb = f32(inputs["node_embeddings"])
    time, day = f32(inputs["time"]), f32(inputs["day"])
    speed, occupy = f32(inputs["speed"]), f32(inputs["occupy"])
    xs = np.concatenate([x, state], axis=-1)

    def wz64_prep(wpool):
        # tile t = k*5+ep: rows [w[2ep,k,2:66,:]; w[2ep+1,k,2:66,:]]
        return np.concatenate(
            [wpool[2 * ep + h, k, DIN:C, :]
             for k in range(CHEB) for ep in range(5) for h in (0, 1)],
            axis=0)

    def wz40_prep(wpool):
        # row j*10+e = w[e,k,c,:] with j-order [(1,0),(1,1),(0,0),(0,1)]
        # (matches xq rows [Lx0, Lx1, x0, x1])
        return np.concatenate(
            [wpool[:, k, c, :] for (k, c) in ((1, 0), (1, 1), (0, 0), (0, 1))],
            axis=0)

    shared = {
        "embT": bf(emb.T),
        "embP": bf(np.repeat(emb.T, DOUT, axis=0)),
        "embX": bf(np.tile(emb.T, (2 * CHEB, 1))),
        "wz64_g": bf(wz64_prep(inputs["gate_wpool"])),
        "wz64_u": bf(wz64_prep(inputs["update_wpool"])),
        "wz40_g": bf(wz40_prep(inputs["gate_wpool"])),
        "wz40_u": bf(wz40_prep(inputs["update_wpool"])),
        "sel4": bf(np.repeat(np.eye(2 * DIN, dtype=np.float32), E, axis=1)),
        "bpool_g": bf(inputs["gate_bpool"]),
        "bpool_u": bf(inputs["update_bpool"]),
        "ident": np.eye(128, dtype=np.float32),
    }
    for br, pre in (("g", "gate"), ("u", "update")):
        shared[f"fc1w_{br}"] = bf(inputs[f"{pre}_fc1_w"])
        shared[f"fc2w_{br}"] = bf(inputs[f"{pre}_fc2_w"])
        shared[f"fc3w_{br}"] = bf(inputs[f"{pre}_fc3_w"])
        shared[f"fcb1_{br}"] = f32(inputs[f"{pre}_fc1_b"].reshape(16, 1))
        shared[f"fcb2_{br}"] = f32(inputs[f"{pre}_fc2_b"].reshape(2, 1))
        shared[f"fcb3_{br}"] = f32(inputs[f"{pre}_fc3_b"].reshape(E, 1))

    in_maps = []
    for c in range(NCORES):
        sl = slice(c * BL, (c + 1) * BL)
        m = dict(shared)
        m["x"] = bf(x[sl])
        m["state"] = bf(state[sl])
        m["xsT"] = bf(xs[sl].transpose(0, 2, 1))
        xsT_c = xs[sl].transpose(0, 2, 1)
        m["xsT2"] = bf(np.concatenate([xsT_c[:, DIN:C, :]] * 2, axis=1))
        m["stateT"] = bf(state[sl].transpose(0, 2, 1))
        m["tT"] = bf(time[sl].transpose(0, 2, 1))
        m["dT"] = bf(day[sl].transpose(0, 2, 1))
        m["sT"] = bf(speed[sl].transpose(0, 2, 1))
        m["oT"] = bf(occupy[sl].transpose(0, 2, 1))
        in_maps.append(m)
    return in_maps


def _run(inputs, trace=False):
    nc = _get_nc()
    in_maps = _make_in_maps(inputs)
    res = run_bass_kernel_spmd(nc, in_maps, core_ids=list(range(NCORES)), trace=trace)
    out = np.concatenate(
        [np.asarray(res.results[i]["out"]).transpose(0, 2, 1) for i in range(NCORES)],
        axis=0,
    )
    return out.astype(np.float32), res


def kernel(**inputs):
    out, _ = _run(inputs, trace=False)
    return out

